# revision 14
# baseline (speedup 1.0000x reference)
"""Trainium2 Bass kernel for nn_Attention_7009386627377.

Multi-head attention (16 heads, d=64) over [4, 2048, 1024] hidden states,
sharded across 8 NeuronCores as (batch b = core//2, head-group g = core%2 of
8 heads). Each core computes its disjoint [2048, 512] output slice with no
collectives; the host reassembles [4, 2048, 16, 64].

Per-core pipeline (fp16 compute, fp32 PSUM accumulation):
  DMA priority: wk -> hidden quad0 -> (wq, wv on SWDGE) -> hidden rest, so
  attention group (0,0) starts ~20us in; transposes/K-projections for quads
  1-3 and the remaining Q/K projections are injected between attention steps.
  qT is pre-scaled by EXPC1 so the DVE exp2 bit-trick is ONE tensor_scalar
  (i16 = sc + EXPC2, bitcast fp16); half of all kt-pairs take that path, the
  rest use ScalarE exp ACTIVATE (scale folded). Row sums ride 4-up packed
  PE matmuls on PAIR-ADDED prob tiles (gpsimd adds), halving their PE cost.
  Groups are software-pipelined in one flat (group, kt) stream with lag 2 so
  the next group's scores overlap the previous group's drain+close. Closes
  go through a DRAM xbar-transpose round trip except the last group, which
  transposes on the PE to shorten the tail.
"""
import threading

import numpy as np

B = 4
S = 2048
HID = 1024
JC = 512          # per-core qkv columns = 8 heads x 64
D = 64
N_CORES = 8

LOG2E = 1.4426950408889634
EXPC1 = 0.125 * LOG2E * 1024.0          # folded into qT at projection time
EXPC2 = 15360.0 - 44.0                  # fp16 exponent bias field + offset
SCALE_S = 0.125 / EXPC1                 # ScalarE exp scale (sc -> s_raw/8)

_LOCK = threading.Lock()
_CACHE = {}


def _build(s=S):
    from contextlib import ExitStack

    from concourse import bacc, mybir
    import concourse.bass as bass
    import concourse.tile as tile
    from concourse.masks import make_identity

    F32 = mybir.dt.float32
    F16 = mybir.dt.float16
    I16 = mybir.dt.int16
    EXP = mybir.ActivationFunctionType.Exp
    COPY = mybir.ActivationFunctionType.Copy
    MUL = mybir.AluOpType.mult
    ADD = mybir.AluOpType.add
    SUB = mybir.AluOpType.subtract

    nst = s // 128           # s-tiles
    nq = max(1, s // 512)    # 512-wide quarters of s
    qw = s // nq             # quarter width
    nkt = s // 128           # key tiles

    nc = bacc.Bacc("TRN2", target_bir_lowering=False, debug=False,
                   enable_asserts=False)

    hid = nc.dram_tensor("hidden", [s, HID], F32, kind="ExternalInput").ap()
    msk = nc.dram_tensor("mask", [s, 1], F32, kind="ExternalInput").ap()
    wq_d = nc.dram_tensor("wq", [HID, JC], F32, kind="ExternalInput").ap()
    wk_d = nc.dram_tensor("wk", [HID, JC], F32, kind="ExternalInput").ap()
    wv_d = nc.dram_tensor("wv", [HID, JC], F32, kind="ExternalInput").ap()
    bq_d = nc.dram_tensor("bq", [JC, 1], F32, kind="ExternalInput").ap()
    bk_d = nc.dram_tensor("bk", [JC, 1], F32, kind="ExternalInput").ap()
    bv_d = nc.dram_tensor("bv", [1, JC], F32, kind="ExternalInput").ap()
    out_d = nc.dram_tensor("out", [s, JC], F32, kind="ExternalOutput").ap()

    with tile.TileContext(nc) as tc, ExitStack() as ctx:
        P = ctx.enter_context
        persist = P(tc.tile_pool(name="persist", bufs=1))
        dram_pool = P(tc.tile_pool(name="dram", bufs=1, space="DRAM"))
        hstage_pool = P(tc.tile_pool(name="hstage", bufs=6))
        hbf_pool = P(tc.tile_pool(name="hbf", bufs=5))
        wstage_pool = P(tc.tile_pool(name="wstage", bufs=8))
        pt_pool = P(tc.tile_pool(name="pt", bufs=6))
        padd_pool = P(tc.tile_pool(name="padd", bufs=2))
        ctx_sb_pool = P(tc.tile_pool(name="ctxsb", bufs=2))
        sums_sb_pool = P(tc.tile_pool(name="sumssb", bufs=2))
        outt_pool = P(tc.tile_pool(name="outt", bufs=4))
        outf_pool = P(tc.tile_pool(name="outf", bufs=2))
        # PSUM: "big" = [128,1024] f32 scores slots (2 banks each, bufs=2),
        # "small" = [128,512] f32 slots for ctx/sums/V/transposes (4 banks).
        ps_big = P(tc.tile_pool(name="psbig", bufs=2, space="PSUM"))
        ps_small = P(tc.tile_pool(name="pssmall", bufs=4, space="PSUM"))

        ident = persist.tile([128, 128], F16, tag="ident")
        make_identity(nc, ident[:])
        ones_rep = persist.tile([128, 32], F16, tag="ones_rep")
        nc.vector.memset(ones_rep[:], 1.0)
        ones_row = persist.tile([1, 128], F16, tag="ones_row")
        nc.vector.memset(ones_row[:], 1.0)

        # ---- DMA issue (sync/scalar = HWDGE, gpsimd = SWDGE) ----
        # priority: wk + hidden quad0 interleaved, then hidden rest on
        # HWDGE; wq -> wv on SWDGE.
        w_stage = {}
        h_stage = []
        for hc in range(8):
            st_t = wstage_pool.tile([128, JC], F32, tag="wstage",
                                    name=f"wk_st{hc}")
            (nc.sync if hc % 2 == 0 else nc.scalar).dma_start(
                st_t[:], wk_d[hc * 128:(hc + 1) * 128, :])
            w_stage[("wk", hc)] = st_t
            if hc % 2 == 0:
                t = hc // 2
                hs = hstage_pool.tile([128, HID], F32, tag="hs",
                                      name=f"hs{t}")
                nc.scalar.dma_start(hs[:], hid[t * 128:(t + 1) * 128, :])
                h_stage.append(hs)

        for t in range(4, nst):
            hs = hstage_pool.tile([128, HID], F32, tag="hs", name=f"hs{t}")
            nc.sync.dma_start(hs[:], hid[t * 128:(t + 1) * 128, :])
            h_stage.append(hs)

        for wname, wd in (("wq", wq_d), ("wv", wv_d)):
            for hc in range(8):
                st_t = wstage_pool.tile([128, JC], F32, tag="wstage",
                                        name=f"{wname}_st{hc}")
                nc.gpsimd.dma_start(st_t[:],
                                    wd[hc * 128:(hc + 1) * 128, :])
                w_stage[(wname, hc)] = st_t

        # mask [s,1] -> [128, nst]; biases as per-partition columns
        mask_sb = persist.tile([128, nst], F32, tag="mask_sb")
        for t in range(nst):
            nc.scalar.dma_start(mask_sb[:, t:t + 1],
                                msk[t * 128:(t + 1) * 128, :])
        bq_sb = persist.tile([128, 4], F32, tag="bq_sb")
        bk_sb = persist.tile([128, 4], F32, tag="bk_sb")
        for p in range(4):
            nc.scalar.dma_start(bq_sb[:, p:p + 1],
                                bq_d[p * 128:(p + 1) * 128, :])
            nc.scalar.dma_start(bk_sb[:, p:p + 1],
                                bk_d[p * 128:(p + 1) * 128, :])
        bv_st = persist.tile([1, JC], F32, tag="bv_st")
        nc.scalar.dma_start(bv_st[:], bv_d[:, :])
        bv_f16 = persist.tile([1, JC], F16, tag="bv_f16")
        nc.vector.tensor_copy(bv_f16[:], bv_st[:])
        # mb = (mask-1)*30: additive exp bias column per kt (0 for mask=1)
        mb = persist.tile([128, nst], F32, tag="mb")
        nc.vector.tensor_scalar(mb[:], mask_sb[:], 1.0, 30.0, SUB, MUL)

        # weight fp16 copies: wk on vector (needed first), wq/wv on scalar
        w_sb = {}
        for wname, eng in (("wk", nc.vector), ("wq", nc.scalar),
                           ("wv", nc.scalar)):
            for hc in range(8):
                wt = persist.tile([128, JC], F16, tag=f"{wname}{hc}")
                if eng is nc.scalar:
                    nc.scalar.activation(wt[:], w_stage[(wname, hc)][:], COPY)
                else:
                    eng.tensor_copy(wt[:], w_stage[(wname, hc)][:])
                w_sb[(wname, hc)] = wt

        hT = [persist.tile([128, s], F16, tag=f"hT{hc}", name=f"hT{hc}")
              for hc in range(8)]
        qT = [persist.tile([128, s], F16, tag=f"qT{p}", name=f"qT{p}")
              for p in range(4)]
        kT = [persist.tile([128, s], F16, tag=f"kT{p}", name=f"kT{p}")
              for p in range(4)]
        v_sb = [persist.tile([128, JC], F16, tag=f"v{t}", name=f"v{t}")
                for t in range(nst)]
        scratch = dram_pool.tile([544, s], F16, tag="scratch")

        zrow = persist.tile([16, 512], F16, tag="zrow")
        nc.vector.memset(zrow[:], 0.0)
        for g in range(2):
            for zc in range(s // 512):
                nc.gpsimd.dma_start(
                    scratch[272 * g + 260:272 * g + 272,
                            zc * 512:(zc + 1) * 512], zrow[0:12, :])

        hb_tiles = {}

        def produce_hb(sq):
            # f32 -> fp16 staging copies for a quad's 4 s-tiles
            for j in range(4):
                hb = hbf_pool.tile([128, HID], F16, tag="hb",
                                   name=f"hb{sq}_{j}")
                nc.vector.tensor_copy(hb[:], h_stage[4 * sq + j][:])
                hb_tiles[4 * sq + j] = hb

        def produce_ht(sq, hcs):
            # transposes for s-tiles of quad sq, head-chunks hcs, via regular
            # matmuls; one [128,512] psum per hc holds the 4 st transposes
            for hc in hcs:
                tp = ps_small.tile([128, 512], F32, tag="ps",
                                   name=f"tp{sq}_{hc}")
                for j in range(4):
                    nc.tensor.matmul(tp[:, j * 128:(j + 1) * 128],
                                     lhsT=hb_tiles[4 * sq + j][:,
                                                               hc * 128:(hc + 1) * 128],
                                     rhs=ident[:], start=True, stop=True,
                                     skip_group_check=True)
                nc.vector.tensor_copy(hT[hc][:, sq * 512:(sq + 1) * 512],
                                      tp[:])

        def project_k(p, sq):
            pp = ps_small.tile([128, qw], F32, tag="ps", name=f"ppk{p}_{sq}")
            for hc in range(8):
                nc.tensor.matmul(
                    pp[:], lhsT=w_sb[("wk", hc)][:, p * 128:(p + 1) * 128],
                    rhs=hT[hc][:, sq * qw:(sq + 1) * qw],
                    start=(hc == 0), stop=(hc == 7))
            nc.vector.tensor_scalar(kT[p][:, sq * qw:(sq + 1) * qw],
                                    pp[:], bk_sb[:, p:p + 1], None, ADD)

        def project_q(p, sq):
            # bias add + EXPC1 prescale folded into the psum->sbuf copy
            pp = ps_small.tile([128, qw], F32, tag="ps", name=f"ppq{p}_{sq}")
            for hc in range(8):
                nc.tensor.matmul(
                    pp[:], lhsT=w_sb[("wq", hc)][:, p * 128:(p + 1) * 128],
                    rhs=hT[hc][:, sq * qw:(sq + 1) * qw],
                    start=(hc == 0), stop=(hc == 7))
            nc.vector.tensor_scalar(qT[p][:, sq * qw:(sq + 1) * qw],
                                    pp[:], bq_sb[:, p:p + 1], EXPC1,
                                    ADD, MUL)

        def produce_v(st):
            # V for s-tile st (+bias via K=1 matmul, mask fold on the copy)
            vp = ps_small.tile([128, JC], F32, tag="ps", name=f"vp{st}")
            for hc in range(8):
                nc.tensor.matmul(vp[:],
                                 lhsT=hT[hc][:, st * 128:(st + 1) * 128],
                                 rhs=w_sb[("wv", hc)][:],
                                 start=(hc == 0), stop=False)
            nc.tensor.matmul(vp[:], lhsT=ones_row[:], rhs=bv_f16[:],
                             start=False, stop=True)
            nc.scalar.activation(v_sb[st][:], vp[:], COPY,
                                 scale=mask_sb[:, st:st + 1])

        # ---- attention: flat (group, kt) stream, lag-2 pipeline ----
        class Group:
            def __init__(g, q, r):
                g.q, g.r = q, r
                g.qs = slice(q * qw, (q + 1) * qw)
                g.pA, g.pB = 2 * r, 2 * r + 1
                g.ctxA = None
                g.prev = None

            def alloc(g):
                g.ctxA = ps_small.tile([128, qw], F32, tag="ps",
                                       name=f"ctxA{g.q}_{g.r}")
                g.ctxB = ps_small.tile([128, qw], F32, tag="ps",
                                       name=f"ctxB{g.q}_{g.r}")
                g.sums = ps_small.tile([128, qw], F32, tag="ps",
                                       name=f"sums{g.q}_{g.r}")

            def scores_exp(g, kt):
                ks = slice(kt * 128, (kt + 1) * 128)
                # one [128, 2048] pt tile per step: halves for head pairs
                # A/B, so the later pair-add is a single DVE op
                ptt = pt_pool.tile([128, 4 * qw], F16, tag="pt")
                pts = []
                for i, ppp in enumerate((g.pA, g.pB)):
                    sc = ps_big.tile([128, 2 * qw], F32, tag="big")
                    nc.tensor.matmul(sc[:, 0:qw], lhsT=kT[ppp][0:64, ks],
                                     rhs=qT[ppp][0:64, g.qs],
                                     start=True, stop=True,
                                     skip_group_check=True,
                                     tile_position=(0, 0))
                    nc.tensor.matmul(sc[:, qw:2 * qw],
                                     lhsT=kT[ppp][64:128, ks],
                                     rhs=qT[ppp][64:128, g.qs],
                                     start=True, stop=True,
                                     skip_group_check=True,
                                     tile_position=(64, 0))
                    half = ptt[:, i * 2 * qw:(i + 1) * 2 * qw]
                    if (kt // 2) in (2, 6):
                        # DVE exp2 bit-trick: one ALU op (qT pre-scaled)
                        nc.vector.tensor_scalar(half.bitcast(I16), sc[:],
                                                EXPC2, None, ADD)
                    else:
                        nc.scalar.activation(half, sc[:], EXP,
                                             scale=SCALE_S,
                                             bias=mb[:, kt:kt + 1])
                    pts.append(half)
                return [ptt] + pts

            def ctx_sums(g, kt, ptt, ptA, ptB):
                if g.ctxA is None:
                    g.alloc()
                for ppp, ctx_ps, pt in ((g.pA, g.ctxA, ptA),
                                        (g.pB, g.ctxB, ptB)):
                    nc.tensor.matmul(
                        ctx_ps[0:64, :],
                        lhsT=v_sb[kt][:, ppp * 128:ppp * 128 + 64],
                        rhs=pt[:, 0:qw], start=(kt == 0),
                        stop=(kt == nkt - 1), skip_group_check=True,
                        tile_position=(0, 0))
                    nc.tensor.matmul(
                        ctx_ps[64:128, :],
                        lhsT=v_sb[kt][:, ppp * 128 + 64:ppp * 128 + 128],
                        rhs=pt[:, qw:2 * qw], start=(kt == 0),
                        stop=(kt == nkt - 1), skip_group_check=True,
                        tile_position=(0, 64))
                if kt % 2 == 0:
                    g.prev = ptt
                    return
                # pair-added prob tiles halve the 4-up sums matmul rate;
                # one [128, 4*qw] DVE add covers both head pairs
                pa = padd_pool.tile([128, 4 * qw], F16, tag="padd")
                nc.vector.tensor_tensor(pa[:], g.prev[:], ptt[:], ADD)
                g.prev = None
                j = kt // 2
                for i, pa_half in enumerate(
                        (pa[:, 0:qw], pa[:, qw:2 * qw],
                         pa[:, 2 * qw:3 * qw], pa[:, 3 * qw:4 * qw])):
                    nc.tensor.matmul(
                        g.sums[32 * i:32 * (i + 1), :], lhsT=ones_rep[:],
                        rhs=pa_half, start=(j == 0),
                        stop=(j == nkt // 2 - 1), skip_group_check=True,
                        tile_position=(0, 32 * i))

            def close(g):
                q, r = g.q, g.r
                base = 272 * r
                for gi, ctx_ps in ((0, g.ctxA), (1, g.ctxB)):
                    ctx_sb = ctx_sb_pool.tile([128, qw], F16, tag="ctxsb")
                    nc.vector.tensor_copy(ctx_sb[:], ctx_ps[:])
                    nc.sync.dma_start(
                        scratch[base + gi * 128:base + (gi + 1) * 128, g.qs],
                        ctx_sb[:])
                ssb = sums_sb_pool.tile([128, qw], F16, tag="sumssb")
                for i in range(4):
                    nc.vector.tensor_copy(ssb[32 * i:32 * i + 1, :],
                                          g.sums[32 * i:32 * i + 1, :])
                    nc.sync.dma_start(
                        scratch[base + 256 + i:base + 257 + i, g.qs],
                        ssb[32 * i:32 * i + 1, :])
                for b4 in range(qw // 128):
                    sbg = q * (qw // 128) + b4
                    ot = outt_pool.tile([128, 272], F16, tag="outt")
                    (nc.sync if b4 % 2 == 0 else nc.scalar).dma_start_transpose(
                        ot[:], scratch[base:base + 272,
                                       sbg * 128:(sbg + 1) * 128])
                    rc = persist.tile([128, 4], F32, tag=f"rc{sbg}_{r}",
                                      name=f"rc{sbg}_{r}")
                    nc.vector.reciprocal(rc[:], ot[:, 256:260])
                    of = outf_pool.tile([128, 256], F32, tag="outf")
                    for h in range(4):
                        nc.vector.tensor_scalar(
                            of[:, h * D:(h + 1) * D],
                            ot[:, h * D:(h + 1) * D],
                            rc[:, h:h + 1], None, MUL)
                    nc.sync.dma_start(
                        out_d[sbg * 128:(sbg + 1) * 128,
                              r * 256:(r + 1) * 256], of[:])

            def close_fast(g):
                # last group: transpose ctx/sums on the PE instead of the
                # DMA xbar round trip through DRAM — shortens the tail.
                q, r = g.q, g.r
                csA = ctx_sb_pool.tile([128, qw], F16, tag="ctxsb")
                nc.scalar.activation(csA[:], g.ctxA[:], COPY)
                csB = ctx_sb_pool.tile([128, qw], F16, tag="ctxsb")
                nc.vector.tensor_copy(csB[:], g.ctxB[:])
                # zero first: the identity-matmul transpose reads ALL 128
                # rows, and NaN bit-patterns in garbage rows would poison
                # every output column (NaN * 0 = NaN)
                ssb = sums_sb_pool.tile([128, qw], F16, tag="sumssb")
                nc.vector.memset(ssb[:], 0.0)
                for i in range(4):
                    nc.vector.tensor_copy(ssb[32 * i:32 * i + 1, :],
                                          g.sums[32 * i:32 * i + 1, :])
                for b4 in range(qw // 128):
                    sbg = q * (qw // 128) + b4
                    cs = slice(b4 * 128, (b4 + 1) * 128)
                    tpo = ps_big.tile([128, 2 * qw], F32, tag="big")
                    nc.tensor.matmul(tpo[:, 0:128], lhsT=csA[:, cs],
                                     rhs=ident[:], start=True, stop=True,
                                     skip_group_check=True)
                    nc.tensor.matmul(tpo[:, 128:256], lhsT=csB[:, cs],
                                     rhs=ident[:], start=True, stop=True,
                                     skip_group_check=True)
                    nc.tensor.matmul(tpo[:, 512:640], lhsT=ssb[:, cs],
                                     rhs=ident[:], start=True, stop=True,
                                     skip_group_check=True)
                    rc = persist.tile([128, 4], F32, tag=f"rcf{sbg}",
                                      name=f"rcf{sbg}")
                    for i in range(4):
                        nc.vector.reciprocal(
                            rc[:, i:i + 1],
                            tpo[:, 512 + 32 * i:513 + 32 * i])
                    of = outf_pool.tile([128, 256], F32, tag="outf")
                    for h in range(4):
                        nc.vector.tensor_scalar(
                            of[:, h * D:(h + 1) * D],
                            tpo[:, (h % 2) * D + (h // 2) * 128:
                                (h % 2) * D + (h // 2) * 128 + D],
                            rc[:, h:h + 1], None, MUL)
                    nc.sync.dma_start(
                        out_d[sbg * 128:(sbg + 1) * 128,
                              r * 256:(r + 1) * 256], of[:])

        # ---- upfront production: quads 0-1 fill the inbound-DMA window ----
        produce_hb(0)
        produce_ht(0, range(8))
        produce_hb(1)
        for p in range(4):
            project_k(p, 0)
        project_q(0, 0)
        project_q(1, 0)
        produce_ht(1, range(8))
        for p in range(4):
            project_k(p, 1)

        groups = [Group(q, r) for q in range(nq) for r in range(2)]

        # injection schedule: gidx -> kt -> list of thunks
        inject = {gi: {} for gi in range(len(groups))}

        def add(gi, kt, fn, *a):
            inject[gi].setdefault(kt, []).append((fn, a))

        # g00: produce quads 2-3 + their kT[0],[1]; Q for g01 at the end
        add(0, 0, produce_hb, 2)
        add(0, 1, produce_ht, 2, range(0, 4))
        add(0, 2, produce_ht, 2, range(4, 8))
        add(0, 3, produce_hb, 3)
        add(0, 4, produce_ht, 3, range(0, 4))
        add(0, 5, produce_ht, 3, range(4, 8))
        add(0, 6, project_k, 0, 2)
        add(0, 7, project_k, 1, 2)
        add(0, 9, project_k, 0, 3)
        add(0, 10, project_k, 1, 3)
        add(0, 13, project_q, 2, 0)
        add(0, 14, project_q, 3, 0)
        # g01: remaining kT[2],[3] quarters 2-3; Q for (1,0)
        add(1, 0, project_k, 2, 2)
        add(1, 1, project_k, 3, 2)
        add(1, 2, project_k, 2, 3)
        add(1, 3, project_k, 3, 3)
        # group (q,0) q>=1 injects Q for (q,1); (q,1) injects Q for (q+1,0)
        for q in range(nq):
            gi_r0, gi_r1 = 2 * q, 2 * q + 1
            if q >= 1:
                add(gi_r0, 5, project_q, 2, q)
                add(gi_r0, 10, project_q, 3, q)
            if q + 1 < nq:
                add(gi_r1, 6, project_q, 0, q + 1)
                add(gi_r1, 10, project_q, 1, q + 1)

        pend = []
        for gi, g in enumerate(groups):
            for kt in range(nkt):
                for fn, a in inject[gi].get(kt, ()):
                    fn(*a)
                if gi == 0:
                    produce_v(kt)
                pts = g.scores_exp(kt)
                pend.append((g, kt, pts))
                if len(pend) > 2:
                    g2, kt2, pts2 = pend.pop(0)
                    g2.ctx_sums(kt2, *pts2)
                    if kt2 == nkt - 1:
                        g2.close()
        while pend:
            g2, kt2, pts2 = pend.pop(0)
            g2.ctx_sums(kt2, *pts2)
            if kt2 == nkt - 1 and g2 is not groups[-1]:
                g2.close()
        groups[-1].close_fast()

    nc.compile()
    return nc


def _get_nc(s=S):
    with _LOCK:
        if s not in _CACHE:
            _CACHE[s] = _build(s)
        return _CACHE[s]


def _make_in_maps(inputs):
    hidden_states = np.asarray(inputs["hidden_states"], dtype=np.float32)
    attention_mask = np.asarray(inputs["attention_mask"], dtype=np.float32)
    Wq = np.asarray(inputs["Wq"], dtype=np.float32)
    Wk = np.asarray(inputs["Wk"], dtype=np.float32)
    Wv = np.asarray(inputs["Wv"], dtype=np.float32)
    bq = np.asarray(inputs["bq"], dtype=np.float32)
    bk = np.asarray(inputs["bk"], dtype=np.float32)
    bv = np.asarray(inputs["bv"], dtype=np.float32)

    in_maps = []
    for core in range(N_CORES):
        b, g = core // 2, core % 2
        js = slice(g * JC, (g + 1) * JC)
        in_maps.append({
            "hidden": np.ascontiguousarray(hidden_states[b]),
            "mask": np.ascontiguousarray(attention_mask[b].reshape(S, 1)),
            "wq": np.ascontiguousarray(Wq[:, js]),
            "wk": np.ascontiguousarray(Wk[:, js]),
            "wv": np.ascontiguousarray(Wv[:, js]),
            "bq": np.ascontiguousarray(bq[js].reshape(JC, 1)),
            "bk": np.ascontiguousarray(bk[js].reshape(JC, 1)),
            "bv": np.ascontiguousarray(bv[js].reshape(1, JC)),
        })
    return in_maps


def kernel(hidden_states, attention_mask, Wq, bq, Wk, bk, Wv, bv):
    from concourse.bass_utils import run_bass_kernel_spmd

    nc = _get_nc()
    in_maps = _make_in_maps(dict(
        hidden_states=hidden_states, attention_mask=attention_mask,
        Wq=Wq, bq=bq, Wk=Wk, bk=bk, Wv=Wv, bv=bv))

    res = run_bass_kernel_spmd(nc, in_maps, core_ids=list(range(N_CORES)))
    out = np.empty((B, S, 16, D), dtype=np.float32)
    for core in range(N_CORES):
        b, g = core // 2, core % 2
        out[b, :, g * 8:(g + 1) * 8, :] = \
            res.results[core]["out"].reshape(S, 8, D)
    return out


# revision 15
# speedup vs baseline: 1.0538x; 1.0538x over previous
"""Trainium2 Bass kernel for nn_Attention_7009386627377.

Multi-head attention (16 heads, d=64) over [4, 2048, 1024] hidden states,
sharded across 8 NeuronCores as (batch b = core//2, head-group g = core%2 of
8 heads). Each core computes its disjoint [2048, 512] output slice with no
collectives; the host reassembles [4, 2048, 16, 64].

Per-core pipeline (fp16 compute, fp32 PSUM accumulation):
  The host pre-casts hidden/weights to fp16 (the device would round them to
  fp16 anyway) and pre-transposes hidden to the [HID, S] layout the PE
  needs, halving inbound DMA and removing all transpose matmuls/staging.
  DMA priority: wk + hT on HWDGE queues, wq -> wv on SWDGE, so attention
  starts ~10us in; K/Q projections stream between attention steps.
  qT is pre-scaled by EXPC1 so the DVE exp2 bit-trick is ONE tensor_scalar
  (i16 = sc + EXPC2, bitcast fp16); kt in {4,5,12,13} take that path, the
  rest use ScalarE exp ACTIVATE (scale folded). Row sums ride 4-up packed
  PE matmuls on PAIR-ADDED prob tiles (one DVE add per kt pair), halving
  their PE cost. Groups are software-pipelined in one flat (group, kt)
  stream with lag 3 so the next group's scores cover the previous group's
  drain+close (keeping the PE HAM-warm across boundaries). Closes go
  through a DRAM xbar-transpose round trip (all on the sync queue) except
  the last group, which transposes on the PE to shorten the tail.
"""
import threading

import numpy as np

B = 4
S = 2048
HID = 1024
JC = 512          # per-core qkv columns = 8 heads x 64
D = 64
N_CORES = 8

LOG2E = 1.4426950408889634
EXPC1 = 0.125 * LOG2E * 1024.0          # folded into qT at projection time
EXPC2 = 15360.0 - 44.0                  # fp16 exponent bias field + offset
SCALE_S = 0.125 / EXPC1                 # ScalarE exp scale (sc -> s_raw/8)
TRICK_KTS = (4, 5, 12, 13)              # kt tiles on the DVE exp2 path

_LOCK = threading.Lock()
_CACHE = {}


def _build(s=S):
    from contextlib import ExitStack

    from concourse import bacc, mybir
    import concourse.bass as bass
    import concourse.tile as tile
    from concourse.masks import make_identity

    F32 = mybir.dt.float32
    F16 = mybir.dt.float16
    I16 = mybir.dt.int16
    EXP = mybir.ActivationFunctionType.Exp
    COPY = mybir.ActivationFunctionType.Copy
    MUL = mybir.AluOpType.mult
    ADD = mybir.AluOpType.add
    SUB = mybir.AluOpType.subtract

    nst = s // 128           # s-tiles
    nq = max(1, s // 512)    # 512-wide quarters of s
    qw = s // nq             # quarter width
    nkt = s // 128           # key tiles

    nc = bacc.Bacc("TRN2", target_bir_lowering=False, debug=False,
                   enable_asserts=False)

    hT_d = nc.dram_tensor("hT", [HID, s], F16, kind="ExternalInput").ap()
    msk = nc.dram_tensor("mask", [s, 1], F32, kind="ExternalInput").ap()
    wq_d = nc.dram_tensor("wq", [HID, JC], F16, kind="ExternalInput").ap()
    wk_d = nc.dram_tensor("wk", [HID, JC], F16, kind="ExternalInput").ap()
    wv_d = nc.dram_tensor("wv", [HID, JC], F16, kind="ExternalInput").ap()
    bq_d = nc.dram_tensor("bq", [JC, 1], F32, kind="ExternalInput").ap()
    bk_d = nc.dram_tensor("bk", [JC, 1], F32, kind="ExternalInput").ap()
    bv_d = nc.dram_tensor("bv", [1, JC], F32, kind="ExternalInput").ap()
    out_d = nc.dram_tensor("out", [s, JC], F32, kind="ExternalOutput").ap()

    with tile.TileContext(nc) as tc, ExitStack() as ctx:
        P = ctx.enter_context
        persist = P(tc.tile_pool(name="persist", bufs=1))
        dram_pool = P(tc.tile_pool(name="dram", bufs=1, space="DRAM"))
        pt_pool = P(tc.tile_pool(name="pt", bufs=7))
        padd_pool = P(tc.tile_pool(name="padd", bufs=2))
        ctx_sb_pool = P(tc.tile_pool(name="ctxsb", bufs=2))
        sums_sb_pool = P(tc.tile_pool(name="sumssb", bufs=2))
        outt_pool = P(tc.tile_pool(name="outt", bufs=4))
        outf_pool = P(tc.tile_pool(name="outf", bufs=2))
        # PSUM: "big" = [128,1024] f32 scores slots (2 banks each, bufs=2),
        # "small" = [128,512] f32 slots for ctx/sums/V/projections (4 banks)
        ps_big = P(tc.tile_pool(name="psbig", bufs=2, space="PSUM"))
        ps_small = P(tc.tile_pool(name="pssmall", bufs=4, space="PSUM"))

        ident = persist.tile([128, 128], F16, tag="ident")
        make_identity(nc, ident[:])
        ones_rep = persist.tile([128, 32], F16, tag="ones_rep")
        nc.vector.memset(ones_rep[:], 1.0)
        ones_row = persist.tile([1, 128], F16, tag="ones_row")
        nc.vector.memset(ones_row[:], 1.0)

        # ---- DMA issue (sync/scalar = HWDGE, gpsimd = SWDGE) ----
        # wk + hT interleaved across both HWDGE queues; wq -> wv on SWDGE
        w_sb = {}
        hT = [persist.tile([128, s], F16, tag=f"hT{hc}", name=f"hT{hc}")
              for hc in range(8)]
        for hc in range(8):
            wt = persist.tile([128, JC], F16, tag=f"wk{hc}")
            (nc.sync if hc % 2 == 0 else nc.scalar).dma_start(
                wt[:], wk_d[hc * 128:(hc + 1) * 128, :])
            w_sb[("wk", hc)] = wt
        for hc in range(8):
            (nc.sync if hc % 2 == 0 else nc.scalar).dma_start(
                hT[hc][:], hT_d[hc * 128:(hc + 1) * 128, :])
        for wname, wd in (("wq", wq_d), ("wv", wv_d)):
            for hc in range(8):
                wt = persist.tile([128, JC], F16, tag=f"{wname}{hc}")
                nc.gpsimd.dma_start(wt[:], wd[hc * 128:(hc + 1) * 128, :])
                w_sb[(wname, hc)] = wt

        # mask [s,1] -> [128, nst]; biases as per-partition columns
        mask_sb = persist.tile([128, nst], F32, tag="mask_sb")
        for t in range(nst):
            nc.scalar.dma_start(mask_sb[:, t:t + 1],
                                msk[t * 128:(t + 1) * 128, :])
        bq_sb = persist.tile([128, 4], F32, tag="bq_sb")
        bk_sb = persist.tile([128, 4], F32, tag="bk_sb")
        for p in range(4):
            nc.scalar.dma_start(bq_sb[:, p:p + 1],
                                bq_d[p * 128:(p + 1) * 128, :])
            nc.scalar.dma_start(bk_sb[:, p:p + 1],
                                bk_d[p * 128:(p + 1) * 128, :])
        bv_st = persist.tile([1, JC], F32, tag="bv_st")
        nc.scalar.dma_start(bv_st[:], bv_d[:, :])
        bv_f16 = persist.tile([1, JC], F16, tag="bv_f16")
        nc.vector.tensor_copy(bv_f16[:], bv_st[:])
        # mb = (mask-1)*30: additive exp bias column per kt (0 for mask=1)
        mb = persist.tile([128, nst], F32, tag="mb")
        nc.vector.tensor_scalar(mb[:], mask_sb[:], 1.0, 30.0, SUB, MUL)

        qT = [persist.tile([128, s], F16, tag=f"qT{p}", name=f"qT{p}")
              for p in range(4)]
        kT = [persist.tile([128, s], F16, tag=f"kT{p}", name=f"kT{p}")
              for p in range(4)]
        v_sb = [persist.tile([128, JC], F16, tag=f"v{t}", name=f"v{t}")
                for t in range(nst)]
        scratch = dram_pool.tile([544, s], F16, tag="scratch")

        zrow = persist.tile([16, 512], F16, tag="zrow")
        nc.vector.memset(zrow[:], 0.0)
        for g in range(2):
            for zc in range(s // 512):
                nc.gpsimd.dma_start(
                    scratch[272 * g + 260:272 * g + 272,
                            zc * 512:(zc + 1) * 512], zrow[0:12, :])

        def project_k(p, sq):
            pp = ps_small.tile([128, qw], F32, tag="ps", name=f"ppk{p}_{sq}")
            for hc in range(8):
                nc.tensor.matmul(
                    pp[:], lhsT=w_sb[("wk", hc)][:, p * 128:(p + 1) * 128],
                    rhs=hT[hc][:, sq * qw:(sq + 1) * qw],
                    start=(hc == 0), stop=(hc == 7))
            nc.vector.tensor_scalar(kT[p][:, sq * qw:(sq + 1) * qw],
                                    pp[:], bk_sb[:, p:p + 1], None, ADD)

        def project_q(p, sq):
            # bias add + EXPC1 prescale folded into the psum->sbuf copy
            pp = ps_small.tile([128, qw], F32, tag="ps", name=f"ppq{p}_{sq}")
            for hc in range(8):
                nc.tensor.matmul(
                    pp[:], lhsT=w_sb[("wq", hc)][:, p * 128:(p + 1) * 128],
                    rhs=hT[hc][:, sq * qw:(sq + 1) * qw],
                    start=(hc == 0), stop=(hc == 7))
            nc.vector.tensor_scalar(qT[p][:, sq * qw:(sq + 1) * qw],
                                    pp[:], bq_sb[:, p:p + 1], EXPC1,
                                    ADD, MUL)

        def produce_v(st):
            # V for s-tile st (+bias via K=1 matmul, mask fold on the copy)
            vp = ps_small.tile([128, JC], F32, tag="ps", name=f"vp{st}")
            for hc in range(8):
                nc.tensor.matmul(vp[:],
                                 lhsT=hT[hc][:, st * 128:(st + 1) * 128],
                                 rhs=w_sb[("wv", hc)][:],
                                 start=(hc == 0), stop=False)
            nc.tensor.matmul(vp[:], lhsT=ones_row[:], rhs=bv_f16[:],
                             start=False, stop=True)
            nc.scalar.activation(v_sb[st][:], vp[:], COPY,
                                 scale=mask_sb[:, st:st + 1])

        # ---- attention: flat (group, kt) stream, lag-3 pipeline ----
        class Group:
            def __init__(g, q, r):
                g.q, g.r = q, r
                g.qs = slice(q * qw, (q + 1) * qw)
                g.pA, g.pB = 2 * r, 2 * r + 1
                g.ctxA = None
                g.prev = None

            def alloc(g):
                g.ctxA = ps_small.tile([128, qw], F32, tag="ps",
                                       name=f"ctxA{g.q}_{g.r}")
                g.ctxB = ps_small.tile([128, qw], F32, tag="ps",
                                       name=f"ctxB{g.q}_{g.r}")
                g.sums = ps_small.tile([128, qw], F32, tag="ps",
                                       name=f"sums{g.q}_{g.r}")

            def scores_exp(g, kt):
                ks = slice(kt * 128, (kt + 1) * 128)
                # one [128, 2048] pt tile per step (halves = head pairs
                # A/B) so the later pair-add is a single DVE op. Tricked
                # kts allocate i16 and are bitcast-read as fp16.
                tricked = kt in TRICK_KTS
                ptt = pt_pool.tile([128, 4 * qw], I16 if tricked else F16,
                                   tag="pt")
                pts = []
                for i, ppp in enumerate((g.pA, g.pB)):
                    sc = ps_big.tile([128, 2 * qw], F32, tag="big")
                    nc.tensor.matmul(sc[:, 0:qw], lhsT=kT[ppp][0:64, ks],
                                     rhs=qT[ppp][0:64, g.qs],
                                     start=True, stop=True,
                                     skip_group_check=True,
                                     tile_position=(0, 0))
                    nc.tensor.matmul(sc[:, qw:2 * qw],
                                     lhsT=kT[ppp][64:128, ks],
                                     rhs=qT[ppp][64:128, g.qs],
                                     start=True, stop=True,
                                     skip_group_check=True,
                                     tile_position=(64, 0))
                    half = ptt[:, i * 2 * qw:(i + 1) * 2 * qw]
                    if tricked:
                        # DVE exp2 bit-trick: one ALU op (qT pre-scaled)
                        nc.vector.tensor_scalar(half, sc[:], EXPC2, None,
                                                ADD)
                        pts.append(half.bitcast(F16))
                    else:
                        nc.scalar.activation(half, sc[:], EXP,
                                             scale=SCALE_S,
                                             bias=mb[:, kt:kt + 1])
                        pts.append(half)
                return [ptt, *pts]

            def ctx_sums(g, kt, ptt, ptA, ptB):
                if g.ctxA is None:
                    g.alloc()
                for ppp, ctx_ps, pt in ((g.pA, g.ctxA, ptA),
                                        (g.pB, g.ctxB, ptB)):
                    nc.tensor.matmul(
                        ctx_ps[0:64, :],
                        lhsT=v_sb[kt][:, ppp * 128:ppp * 128 + 64],
                        rhs=pt[:, 0:qw], start=(kt == 0),
                        stop=(kt == nkt - 1), skip_group_check=True,
                        tile_position=(0, 0))
                    nc.tensor.matmul(
                        ctx_ps[64:128, :],
                        lhsT=v_sb[kt][:, ppp * 128 + 64:ppp * 128 + 128],
                        rhs=pt[:, qw:2 * qw], start=(kt == 0),
                        stop=(kt == nkt - 1), skip_group_check=True,
                        tile_position=(0, 64))
                if kt % 2 == 0:
                    g.prev = ptt
                    return
                # pair-added prob tiles halve the 4-up sums matmul rate;
                # one [128, 4*qw] DVE add covers both head pairs
                pa = padd_pool.tile([128, 4 * qw], F16, tag="padd")
                nc.vector.tensor_tensor(
                    pa[:], g.prev[:].bitcast(F16), ptt[:].bitcast(F16), ADD)
                g.prev = None
                j = kt // 2
                for i, pa_half in enumerate(
                        (pa[:, 0:qw], pa[:, qw:2 * qw],
                         pa[:, 2 * qw:3 * qw], pa[:, 3 * qw:4 * qw])):
                    nc.tensor.matmul(
                        g.sums[32 * i:32 * (i + 1), :], lhsT=ones_rep[:],
                        rhs=pa_half, start=(j == 0),
                        stop=(j == nkt // 2 - 1), skip_group_check=True,
                        tile_position=(0, 32 * i))

            def close(g):
                q, r = g.q, g.r
                base = 272 * r
                for gi, ctx_ps in ((0, g.ctxA), (1, g.ctxB)):
                    ctx_sb = ctx_sb_pool.tile([128, qw], F16, tag="ctxsb")
                    nc.vector.tensor_copy(ctx_sb[:], ctx_ps[:])
                    nc.sync.dma_start(
                        scratch[base + gi * 128:base + (gi + 1) * 128, g.qs],
                        ctx_sb[:])
                ssb = sums_sb_pool.tile([128, qw], F16, tag="sumssb")
                for i in range(4):
                    nc.vector.tensor_copy(ssb[32 * i:32 * i + 1, :],
                                          g.sums[32 * i:32 * i + 1, :])
                    nc.sync.dma_start(
                        scratch[base + 256 + i:base + 257 + i, g.qs],
                        ssb[32 * i:32 * i + 1, :])
                for b4 in range(qw // 128):
                    sbg = q * (qw // 128) + b4
                    ot = outt_pool.tile([128, 272], F16, tag="outt")
                    nc.sync.dma_start_transpose(
                        ot[:], scratch[base:base + 272,
                                       sbg * 128:(sbg + 1) * 128])
                    rc = persist.tile([128, 4], F32, tag=f"rc{sbg}_{r}",
                                      name=f"rc{sbg}_{r}")
                    nc.vector.reciprocal(rc[:], ot[:, 256:260])
                    of = outf_pool.tile([128, 256], F32, tag="outf")
                    for h in range(4):
                        nc.vector.tensor_scalar(
                            of[:, h * D:(h + 1) * D],
                            ot[:, h * D:(h + 1) * D],
                            rc[:, h:h + 1], None, MUL)
                    nc.sync.dma_start(
                        out_d[sbg * 128:(sbg + 1) * 128,
                              r * 256:(r + 1) * 256], of[:])

            def close_fast(g):
                # last group: transpose ctx/sums on the PE instead of the
                # DMA xbar round trip through DRAM — shortens the tail.
                q, r = g.q, g.r
                csA = ctx_sb_pool.tile([128, qw], F16, tag="ctxsb")
                nc.scalar.activation(csA[:], g.ctxA[:], COPY)
                csB = ctx_sb_pool.tile([128, qw], F16, tag="ctxsb")
                nc.vector.tensor_copy(csB[:], g.ctxB[:])
                # zero first: the identity-matmul transpose reads ALL 128
                # rows, and NaN bit-patterns in garbage rows would poison
                # every output column (NaN * 0 = NaN)
                ssb = sums_sb_pool.tile([128, qw], F16, tag="sumssb")
                nc.vector.memset(ssb[:], 0.0)
                for i in range(4):
                    nc.vector.tensor_copy(ssb[32 * i:32 * i + 1, :],
                                          g.sums[32 * i:32 * i + 1, :])
                for b4 in range(qw // 128):
                    sbg = q * (qw // 128) + b4
                    cs = slice(b4 * 128, (b4 + 1) * 128)
                    tpo = ps_big.tile([128, 2 * qw], F32, tag="big")
                    nc.tensor.matmul(tpo[:, 0:128], lhsT=csA[:, cs],
                                     rhs=ident[:], start=True, stop=True,
                                     skip_group_check=True)
                    nc.tensor.matmul(tpo[:, 128:256], lhsT=csB[:, cs],
                                     rhs=ident[:], start=True, stop=True,
                                     skip_group_check=True)
                    nc.tensor.matmul(tpo[:, 512:640], lhsT=ssb[:, cs],
                                     rhs=ident[:], start=True, stop=True,
                                     skip_group_check=True)
                    rc = persist.tile([128, 4], F32, tag=f"rcf{sbg}",
                                      name=f"rcf{sbg}")
                    for i in range(4):
                        nc.vector.reciprocal(
                            rc[:, i:i + 1],
                            tpo[:, 512 + 32 * i:513 + 32 * i])
                    of = outf_pool.tile([128, 256], F32, tag="outf")
                    for h in range(4):
                        nc.vector.tensor_scalar(
                            of[:, h * D:(h + 1) * D],
                            tpo[:, (h % 2) * D + (h // 2) * 128:
                                (h % 2) * D + (h // 2) * 128 + D],
                            rc[:, h:h + 1], None, MUL)
                    nc.sync.dma_start(
                        out_d[sbg * 128:(sbg + 1) * 128,
                              r * 256:(r + 1) * 256], of[:])

        # ---- upfront: just enough projections for g00's first steps ----
        project_k(0, 0)
        project_k(1, 0)
        project_q(0, 0)
        project_q(1, 0)

        groups = [Group(q, r) for q in range(nq) for r in range(2)]

        # injection schedule: gidx -> kt -> list of thunks
        inject = {gi: {} for gi in range(len(groups))}

        def add(gi, kt, fn, *a):
            inject[gi].setdefault(kt, []).append((fn, a))

        # g00 streams the remaining K projections just ahead of use
        for kt, (p, sq) in enumerate((
                (0, 1), (1, 1), (2, 0), (3, 0),
                (0, 2), (1, 2), (2, 1), (3, 1),
                (0, 3), (1, 3), (2, 2), (3, 2),
                (2, 3), (3, 3))):
            add(0, kt, project_k, p, sq)
        add(0, 13, project_q, 2, 0)
        add(0, 14, project_q, 3, 0)
        # group (q,0) q>=1 injects Q for (q,1); (q,1) injects Q for (q+1,0)
        for q in range(nq):
            gi_r0, gi_r1 = 2 * q, 2 * q + 1
            if q >= 1:
                add(gi_r0, 5, project_q, 2, q)
                add(gi_r0, 10, project_q, 3, q)
            if q + 1 < nq:
                add(gi_r1, 6, project_q, 0, q + 1)
                add(gi_r1, 10, project_q, 1, q + 1)

        pend = []
        for gi, g in enumerate(groups):
            for kt in range(nkt):
                for fn, a in inject[gi].get(kt, ()):
                    fn(*a)
                pts = g.scores_exp(kt)
                if gi == 0:
                    produce_v(kt)
                pend.append((g, kt, pts))
                if len(pend) > 3:
                    g2, kt2, pts2 = pend.pop(0)
                    g2.ctx_sums(kt2, *pts2)
                    if kt2 == nkt - 1:
                        g2.close()
        while pend:
            g2, kt2, pts2 = pend.pop(0)
            g2.ctx_sums(kt2, *pts2)
            if kt2 == nkt - 1 and g2 is not groups[-1]:
                g2.close()
        groups[-1].close_fast()

    nc.compile()
    return nc


def _get_nc(s=S):
    with _LOCK:
        if s not in _CACHE:
            _CACHE[s] = _build(s)
        return _CACHE[s]


def _make_in_maps(inputs):
    hidden_states = np.asarray(inputs["hidden_states"], dtype=np.float32)
    attention_mask = np.asarray(inputs["attention_mask"], dtype=np.float32)
    Wq = np.asarray(inputs["Wq"], dtype=np.float32).astype(np.float16)
    Wk = np.asarray(inputs["Wk"], dtype=np.float32).astype(np.float16)
    Wv = np.asarray(inputs["Wv"], dtype=np.float32).astype(np.float16)
    bq = np.asarray(inputs["bq"], dtype=np.float32)
    bk = np.asarray(inputs["bk"], dtype=np.float32)
    bv = np.asarray(inputs["bv"], dtype=np.float32)

    in_maps = []
    for core in range(N_CORES):
        b, g = core // 2, core % 2
        js = slice(g * JC, (g + 1) * JC)
        in_maps.append({
            # fp16 + [HID, S] layout: the rounding the device would do on
            # load, plus the transpose the PE would otherwise compute
            "hT": np.ascontiguousarray(
                hidden_states[b].astype(np.float16).T),
            "mask": np.ascontiguousarray(attention_mask[b].reshape(S, 1)),
            "wq": np.ascontiguousarray(Wq[:, js]),
            "wk": np.ascontiguousarray(Wk[:, js]),
            "wv": np.ascontiguousarray(Wv[:, js]),
            "bq": np.ascontiguousarray(bq[js].reshape(JC, 1)),
            "bk": np.ascontiguousarray(bk[js].reshape(JC, 1)),
            "bv": np.ascontiguousarray(bv[js].reshape(1, JC)),
        })
    return in_maps


def kernel(hidden_states, attention_mask, Wq, bq, Wk, bk, Wv, bv):
    from concourse.bass_utils import run_bass_kernel_spmd

    nc = _get_nc()
    in_maps = _make_in_maps(dict(
        hidden_states=hidden_states, attention_mask=attention_mask,
        Wq=Wq, bq=bq, Wk=Wk, bk=bk, Wv=Wv, bv=bv))

    res = run_bass_kernel_spmd(nc, in_maps, core_ids=list(range(N_CORES)))
    out = np.empty((B, S, 16, D), dtype=np.float32)
    for core in range(N_CORES):
        b, g = core // 2, core % 2
        out[b, :, g * 8:(g + 1) * 8, :] = \
            res.results[core]["out"].reshape(S, 8, D)
    return out


# revision 19
# speedup vs baseline: 1.0670x; 1.0125x over previous
"""Trainium2 Bass kernel for nn_Attention_7009386627377.

Multi-head attention (16 heads, d=64) over [4, 2048, 1024] hidden states,
sharded across 8 NeuronCores as (batch b = core//2, head-group g = core%2 of
8 heads). Each core computes its disjoint [2048, 512] output slice with no
collectives; the host reassembles [4, 2048, 16, 64].

Per-core pipeline (fp16 compute, fp32 PSUM accumulation):
  The host pre-casts hidden/weights to fp16 (the device would round them to
  fp16 anyway) and pre-transposes hidden to the [HID, S] layout the PE
  needs, halving inbound DMA and removing all transpose matmuls/staging.
  DMA priority: wk + hT on HWDGE queues, wq -> wv on SWDGE, so attention
  starts ~10us in; K/Q projections stream between attention steps.
  qT is pre-scaled by EXPC1 so the DVE exp2 bit-trick is ONE tensor_scalar
  (i16 = sc + EXPC2, bitcast fp16); kt in {4,5,12,13} take that path, the
  rest use ScalarE exp ACTIVATE (scale folded). Row sums ride 4-up packed
  PE matmuls on PAIR-ADDED prob tiles (one DVE add per kt pair), halving
  their PE cost. Groups are software-pipelined in one flat (group, kt)
  stream with lag 3 so the next group's scores cover the previous group's
  drain+close (keeping the PE HAM-warm across boundaries). Closes go
  through a DRAM xbar-transpose round trip (all on the sync queue) except
  the last group, which transposes on the PE to shorten the tail.
"""
import threading

import numpy as np

B = 4
S = 2048
HID = 1024
JC = 512          # per-core qkv columns = 8 heads x 64
D = 64
N_CORES = 8

LOG2E = 1.4426950408889634
EXPC1 = 0.125 * LOG2E * 1024.0          # folded into qT at projection time
EXPC2 = 15360.0 - 44.0                  # fp16 exponent bias field + offset
SCALE_S = 0.125 / EXPC1                 # ScalarE exp scale (sc -> s_raw/8)
TRICK_KTS = (4, 5, 12, 13)              # kt tiles on the DVE exp2 path

_LOCK = threading.Lock()
_CACHE = {}


def _build(s=S):
    from contextlib import ExitStack

    from concourse import bacc, mybir
    import concourse.bass as bass
    import concourse.tile as tile
    from concourse.masks import make_identity

    F32 = mybir.dt.float32
    F16 = mybir.dt.float16
    I16 = mybir.dt.int16
    EXP = mybir.ActivationFunctionType.Exp
    COPY = mybir.ActivationFunctionType.Copy
    MUL = mybir.AluOpType.mult
    ADD = mybir.AluOpType.add
    SUB = mybir.AluOpType.subtract

    nst = s // 128           # s-tiles
    nq = max(1, s // 512)    # 512-wide quarters of s
    qw = s // nq             # quarter width
    nkt = s // 128           # key tiles

    nc = bacc.Bacc("TRN2", target_bir_lowering=False, debug=False,
                   enable_asserts=False)

    hT_d = nc.dram_tensor("hT", [HID, s], F16, kind="ExternalInput").ap()
    msk = nc.dram_tensor("mask", [s, 1], F32, kind="ExternalInput").ap()
    wq_d = nc.dram_tensor("wq", [HID, JC], F16, kind="ExternalInput").ap()
    wk_d = nc.dram_tensor("wk", [HID, JC], F16, kind="ExternalInput").ap()
    wv_d = nc.dram_tensor("wv", [HID, JC], F16, kind="ExternalInput").ap()
    bq_d = nc.dram_tensor("bq", [JC, 1], F32, kind="ExternalInput").ap()
    bk_d = nc.dram_tensor("bk", [JC, 1], F32, kind="ExternalInput").ap()
    bv_d = nc.dram_tensor("bv", [1, JC], F32, kind="ExternalInput").ap()
    out_d = nc.dram_tensor("out", [s, JC], F32, kind="ExternalOutput").ap()

    with tile.TileContext(nc) as tc, ExitStack() as ctx:
        P = ctx.enter_context
        persist = P(tc.tile_pool(name="persist", bufs=1))
        dram_pool = P(tc.tile_pool(name="dram", bufs=1, space="DRAM"))
        pt_pool = P(tc.tile_pool(name="pt", bufs=7))
        padd_pool = P(tc.tile_pool(name="padd", bufs=2))
        ctx_sb_pool = P(tc.tile_pool(name="ctxsb", bufs=2))
        sums_sb_pool = P(tc.tile_pool(name="sumssb", bufs=2))
        outt_pool = P(tc.tile_pool(name="outt", bufs=4))
        outf_pool = P(tc.tile_pool(name="outf", bufs=2))
        # PSUM: "big" = [128,1024] f32 scores slots (2 banks each, bufs=2),
        # "small" = [128,512] f32 slots for ctx/sums/V/projections (4 banks)
        ps_big = P(tc.tile_pool(name="psbig", bufs=2, space="PSUM"))
        ps_small = P(tc.tile_pool(name="pssmall", bufs=4, space="PSUM"))

        ident = persist.tile([128, 128], F16, tag="ident")
        make_identity(nc, ident[:])
        ones_rep = persist.tile([128, 32], F16, tag="ones_rep")
        nc.vector.memset(ones_rep[:], 1.0)
        ones_row = persist.tile([1, 128], F16, tag="ones_row")
        nc.vector.memset(ones_row[:], 1.0)

        # ---- DMA issue (sync/scalar = HWDGE, gpsimd = SWDGE) ----
        # wk + hT interleaved across both HWDGE queues; wq -> wv on SWDGE
        w_sb = {}
        hT = [persist.tile([128, s], F16, tag=f"hT{hc}", name=f"hT{hc}")
              for hc in range(8)]
        for hc in range(8):
            wt = persist.tile([128, JC], F16, tag=f"wk{hc}")
            (nc.sync if hc % 2 == 0 else nc.scalar).dma_start(
                wt[:], wk_d[hc * 128:(hc + 1) * 128, :])
            w_sb[("wk", hc)] = wt
        # quarter-major hT chunks so the first K projections (which contract
        # over ALL hc) can start after ~1MB instead of the full 4.2MB
        for sq in range(nq):
            for hc in range(8):
                (nc.sync if hc % 2 == 0 else nc.scalar).dma_start(
                    hT[hc][:, sq * qw:(sq + 1) * qw],
                    hT_d[hc * 128:(hc + 1) * 128, sq * qw:(sq + 1) * qw])
        for wname, wd in (("wq", wq_d), ("wv", wv_d)):
            for hc in range(8):
                wt = persist.tile([128, JC], F16, tag=f"{wname}{hc}")
                nc.gpsimd.dma_start(wt[:], wd[hc * 128:(hc + 1) * 128, :])
                w_sb[(wname, hc)] = wt

        # mask [s,1] -> [128, nst]; biases as per-partition columns
        mask_sb = persist.tile([128, nst], F32, tag="mask_sb")
        for t in range(nst):
            nc.scalar.dma_start(mask_sb[:, t:t + 1],
                                msk[t * 128:(t + 1) * 128, :])
        bq_sb = persist.tile([128, 4], F32, tag="bq_sb")
        bk_sb = persist.tile([128, 4], F32, tag="bk_sb")
        for p in range(4):
            nc.scalar.dma_start(bq_sb[:, p:p + 1],
                                bq_d[p * 128:(p + 1) * 128, :])
            nc.scalar.dma_start(bk_sb[:, p:p + 1],
                                bk_d[p * 128:(p + 1) * 128, :])
        bv_st = persist.tile([1, JC], F32, tag="bv_st")
        nc.scalar.dma_start(bv_st[:], bv_d[:, :])
        bv_f16 = persist.tile([1, JC], F16, tag="bv_f16")
        nc.vector.tensor_copy(bv_f16[:], bv_st[:])
        # mb = (mask-1)*30: additive exp bias column per kt (0 for mask=1)
        mb = persist.tile([128, nst], F32, tag="mb")
        nc.vector.tensor_scalar(mb[:], mask_sb[:], 1.0, 30.0, SUB, MUL)

        qT = [persist.tile([128, s], F16, tag=f"qT{p}", name=f"qT{p}")
              for p in range(4)]
        kT = [persist.tile([128, s], F16, tag=f"kT{p}", name=f"kT{p}")
              for p in range(4)]
        v_sb = [persist.tile([128, JC], F16, tag=f"v{t}", name=f"v{t}")
                for t in range(nst)]
        scratch = dram_pool.tile([544, s], F16, tag="scratch")

        zrow = persist.tile([16, 512], F16, tag="zrow")
        nc.vector.memset(zrow[:], 0.0)
        for g in range(2):
            for zc in range(s // 512):
                nc.gpsimd.dma_start(
                    scratch[272 * g + 260:272 * g + 272,
                            zc * 512:(zc + 1) * 512], zrow[0:12, :])

        def project_k(p, sq):
            pp = ps_small.tile([128, qw], F32, tag="ps", name=f"ppk{p}_{sq}")
            for hc in range(8):
                nc.tensor.matmul(
                    pp[:], lhsT=w_sb[("wk", hc)][:, p * 128:(p + 1) * 128],
                    rhs=hT[hc][:, sq * qw:(sq + 1) * qw],
                    start=(hc == 0), stop=(hc == 7))
            nc.vector.tensor_scalar(kT[p][:, sq * qw:(sq + 1) * qw],
                                    pp[:], bk_sb[:, p:p + 1], None, ADD)

        def project_q(p, sq):
            # bias add + EXPC1 prescale folded into the psum->sbuf copy
            pp = ps_small.tile([128, qw], F32, tag="ps", name=f"ppq{p}_{sq}")
            for hc in range(8):
                nc.tensor.matmul(
                    pp[:], lhsT=w_sb[("wq", hc)][:, p * 128:(p + 1) * 128],
                    rhs=hT[hc][:, sq * qw:(sq + 1) * qw],
                    start=(hc == 0), stop=(hc == 7))
            nc.vector.tensor_scalar(qT[p][:, sq * qw:(sq + 1) * qw],
                                    pp[:], bq_sb[:, p:p + 1], EXPC1,
                                    ADD, MUL)

        def produce_v(st):
            # V for s-tile st (+bias via K=1 matmul, mask fold on the copy)
            vp = ps_small.tile([128, JC], F32, tag="ps", name=f"vp{st}")
            for hc in range(8):
                nc.tensor.matmul(vp[:],
                                 lhsT=hT[hc][:, st * 128:(st + 1) * 128],
                                 rhs=w_sb[("wv", hc)][:],
                                 start=(hc == 0), stop=False)
            nc.tensor.matmul(vp[:], lhsT=ones_row[:], rhs=bv_f16[:],
                             start=False, stop=True)
            nc.scalar.activation(v_sb[st][:], vp[:], COPY,
                                 scale=mask_sb[:, st:st + 1])

        # ---- attention: flat (group, kt) stream, lag-3 pipeline ----
        class Group:
            def __init__(g, q, r):
                g.q, g.r = q, r
                g.qs = slice(q * qw, (q + 1) * qw)
                g.pA, g.pB = 2 * r, 2 * r + 1
                g.ctxA = None
                g.prev = None

            def alloc(g):
                g.ctxA = ps_small.tile([128, qw], F32, tag="ps",
                                       name=f"ctxA{g.q}_{g.r}")
                g.ctxB = ps_small.tile([128, qw], F32, tag="ps",
                                       name=f"ctxB{g.q}_{g.r}")
                g.sums = ps_small.tile([128, qw], F32, tag="ps",
                                       name=f"sums{g.q}_{g.r}")

            def scores_exp(g, kt):
                ks = slice(kt * 128, (kt + 1) * 128)
                # one [128, 2048] pt tile per step (halves = head pairs
                # A/B) so the later pair-add is a single DVE op. Tricked
                # kts allocate i16 and are bitcast-read as fp16.
                tricked = kt in TRICK_KTS
                ptt = pt_pool.tile([128, 4 * qw], I16 if tricked else F16,
                                   tag="pt")
                pts = []
                for i, ppp in enumerate((g.pA, g.pB)):
                    sc = ps_big.tile([128, 2 * qw], F32, tag="big")
                    nc.tensor.matmul(sc[:, 0:qw], lhsT=kT[ppp][0:64, ks],
                                     rhs=qT[ppp][0:64, g.qs],
                                     start=True, stop=True,
                                     skip_group_check=True,
                                     tile_position=(0, 0))
                    nc.tensor.matmul(sc[:, qw:2 * qw],
                                     lhsT=kT[ppp][64:128, ks],
                                     rhs=qT[ppp][64:128, g.qs],
                                     start=True, stop=True,
                                     skip_group_check=True,
                                     tile_position=(64, 0))
                    half = ptt[:, i * 2 * qw:(i + 1) * 2 * qw]
                    if tricked:
                        # DVE exp2 bit-trick: one ALU op (qT pre-scaled)
                        nc.vector.tensor_scalar(half, sc[:], EXPC2, None,
                                                ADD)
                        pts.append(half.bitcast(F16))
                    else:
                        nc.scalar.activation(half, sc[:], EXP,
                                             scale=SCALE_S,
                                             bias=mb[:, kt:kt + 1])
                        pts.append(half)
                return [ptt, *pts]

            def ctx_sums(g, kt, ptt, ptA, ptB):
                if g.ctxA is None:
                    g.alloc()
                for ppp, ctx_ps, pt in ((g.pA, g.ctxA, ptA),
                                        (g.pB, g.ctxB, ptB)):
                    nc.tensor.matmul(
                        ctx_ps[0:64, :],
                        lhsT=v_sb[kt][:, ppp * 128:ppp * 128 + 64],
                        rhs=pt[:, 0:qw], start=(kt == 0),
                        stop=(kt == nkt - 1), skip_group_check=True,
                        tile_position=(0, 0))
                    nc.tensor.matmul(
                        ctx_ps[64:128, :],
                        lhsT=v_sb[kt][:, ppp * 128 + 64:ppp * 128 + 128],
                        rhs=pt[:, qw:2 * qw], start=(kt == 0),
                        stop=(kt == nkt - 1), skip_group_check=True,
                        tile_position=(0, 64))
                if kt % 2 == 0:
                    g.prev = ptt
                    return
                # pair-added prob tiles halve the 4-up sums matmul rate;
                # one [128, 4*qw] DVE add covers both head pairs
                pa = padd_pool.tile([128, 4 * qw], F16, tag="padd")
                nc.vector.tensor_tensor(
                    pa[:], g.prev[:].bitcast(F16), ptt[:].bitcast(F16), ADD)
                g.prev = None
                j = kt // 2
                for i, pa_half in enumerate(
                        (pa[:, 0:qw], pa[:, qw:2 * qw],
                         pa[:, 2 * qw:3 * qw], pa[:, 3 * qw:4 * qw])):
                    nc.tensor.matmul(
                        g.sums[32 * i:32 * (i + 1), :], lhsT=ones_rep[:],
                        rhs=pa_half, start=(j == 0),
                        stop=(j == nkt // 2 - 1), skip_group_check=True,
                        tile_position=(0, 32 * i))

            def close_write(g):
                # copies + scratch writes + xbar transpose issue; the
                # normalize waits on xbar data, so it's DEFERRED (a few
                # steps later) to keep the in-order DVE from stalling and
                # holding up the next group's padds/exps behind it.
                q, r = g.q, g.r
                base = 272 * r
                for gi, ctx_ps in ((0, g.ctxA), (1, g.ctxB)):
                    ctx_sb = ctx_sb_pool.tile([128, qw], F16, tag="ctxsb")
                    nc.vector.tensor_copy(ctx_sb[:], ctx_ps[:])
                    nc.sync.dma_start(
                        scratch[base + gi * 128:base + (gi + 1) * 128, g.qs],
                        ctx_sb[:])
                ssb = sums_sb_pool.tile([128, qw], F16, tag="sumssb")
                for i in range(4):
                    nc.vector.tensor_copy(ssb[32 * i:32 * i + 1, :],
                                          g.sums[32 * i:32 * i + 1, :])
                    nc.sync.dma_start(
                        scratch[base + 256 + i:base + 257 + i, g.qs],
                        ssb[32 * i:32 * i + 1, :])
                g.ots = []
                for b4 in range(qw // 128):
                    sbg = q * (qw // 128) + b4
                    ot = outt_pool.tile([128, 272], F16, tag="outt")
                    nc.sync.dma_start_transpose(
                        ot[:], scratch[base:base + 272,
                                       sbg * 128:(sbg + 1) * 128])
                    g.ots.append(ot)

            def close_finish(g):
                q, r = g.q, g.r
                for b4 in range(qw // 128):
                    sbg = q * (qw // 128) + b4
                    ot = g.ots[b4]
                    rc = persist.tile([128, 4], F32, tag=f"rc{sbg}_{r}",
                                      name=f"rc{sbg}_{r}")
                    nc.vector.reciprocal(rc[:], ot[:, 256:260])
                    of = outf_pool.tile([128, 256], F32, tag="outf")
                    for h in range(4):
                        nc.vector.tensor_scalar(
                            of[:, h * D:(h + 1) * D],
                            ot[:, h * D:(h + 1) * D],
                            rc[:, h:h + 1], None, MUL)
                    nc.sync.dma_start(
                        out_d[sbg * 128:(sbg + 1) * 128,
                              r * 256:(r + 1) * 256], of[:])

            def close_fast(g):
                # last group: transpose ctx/sums on the PE instead of the
                # DMA xbar round trip through DRAM — shortens the tail.
                q, r = g.q, g.r
                csA = ctx_sb_pool.tile([128, qw], F16, tag="ctxsb")
                nc.scalar.activation(csA[:], g.ctxA[:], COPY)
                csB = ctx_sb_pool.tile([128, qw], F16, tag="ctxsb")
                nc.vector.tensor_copy(csB[:], g.ctxB[:])
                # zero first: the identity-matmul transpose reads ALL 128
                # rows, and NaN bit-patterns in garbage rows would poison
                # every output column (NaN * 0 = NaN)
                ssb = sums_sb_pool.tile([128, qw], F16, tag="sumssb")
                nc.vector.memset(ssb[:], 0.0)
                for i in range(4):
                    nc.vector.tensor_copy(ssb[32 * i:32 * i + 1, :],
                                          g.sums[32 * i:32 * i + 1, :])
                for b4 in range(qw // 128):
                    sbg = q * (qw // 128) + b4
                    cs = slice(b4 * 128, (b4 + 1) * 128)
                    tpo = ps_big.tile([128, 2 * qw], F32, tag="big")
                    nc.tensor.matmul(tpo[:, 0:128], lhsT=csA[:, cs],
                                     rhs=ident[:], start=True, stop=True,
                                     skip_group_check=True)
                    nc.tensor.matmul(tpo[:, 128:256], lhsT=csB[:, cs],
                                     rhs=ident[:], start=True, stop=True,
                                     skip_group_check=True)
                    nc.tensor.matmul(tpo[:, 512:640], lhsT=ssb[:, cs],
                                     rhs=ident[:], start=True, stop=True,
                                     skip_group_check=True)
                    rc = persist.tile([128, 4], F32, tag=f"rcf{sbg}",
                                      name=f"rcf{sbg}")
                    for i in range(4):
                        nc.vector.reciprocal(
                            rc[:, i:i + 1],
                            tpo[:, 512 + 32 * i:513 + 32 * i])
                    of = outf_pool.tile([128, 256], F32, tag="outf")
                    for h in range(4):
                        nc.vector.tensor_scalar(
                            of[:, h * D:(h + 1) * D],
                            tpo[:, (h % 2) * D + (h // 2) * 128:
                                (h % 2) * D + (h // 2) * 128 + D],
                            rc[:, h:h + 1], None, MUL)
                    nc.sync.dma_start(
                        out_d[sbg * 128:(sbg + 1) * 128,
                              r * 256:(r + 1) * 256], of[:])

        # ---- upfront: just enough projections for g00's first steps ----
        project_k(0, 0)
        project_k(1, 0)
        project_q(0, 0)
        project_q(1, 0)

        groups = [Group(q, r) for q in range(nq) for r in range(2)]

        # injection schedule: gidx -> kt -> list of thunks
        inject = {gi: {} for gi in range(len(groups))}

        def add(gi, kt, fn, *a):
            inject[gi].setdefault(kt, []).append((fn, a))

        # g00 streams the remaining K projections just ahead of use
        for kt, (p, sq) in enumerate((
                (0, 1), (1, 1), (2, 0), (3, 0),
                (0, 2), (1, 2), (2, 1), (3, 1),
                (0, 3), (1, 3), (2, 2), (3, 2),
                (2, 3), (3, 3))):
            add(0, kt, project_k, p, sq)
        add(0, 13, project_q, 2, 0)
        add(0, 14, project_q, 3, 0)
        # group (q,0) q>=1 injects Q for (q,1); (q,1) injects Q for (q+1,0)
        for q in range(nq):
            gi_r0, gi_r1 = 2 * q, 2 * q + 1
            if q >= 1:
                add(gi_r0, 5, project_q, 2, q)
                add(gi_r0, 10, project_q, 3, q)
            if q + 1 < nq:
                add(gi_r1, 6, project_q, 0, q + 1)
                add(gi_r1, 10, project_q, 1, q + 1)

        pend = []
        todo = []

        def tick_todo():
            for item in list(todo):
                item[0] -= 1
                if item[0] <= 0:
                    todo.remove(item)
                    item[1]()

        def drain_one():
            g2, kt2, pts2 = pend.pop(0)
            g2.ctx_sums(kt2, *pts2)
            if kt2 == nkt - 1 and g2 is not groups[-1]:
                g2.close_write()
                todo.append([3, lambda gg=g2: gg.close_finish()])

        for gi, g in enumerate(groups):
            for kt in range(nkt):
                for fn, a in inject[gi].get(kt, ()):
                    fn(*a)
                pts = g.scores_exp(kt)
                if gi == 0:
                    produce_v(kt)
                pend.append((g, kt, pts))
                tick_todo()
                if len(pend) > 3:
                    drain_one()
        while pend:
            drain_one()
        for item in todo:
            item[1]()
        groups[-1].close_fast()

    nc.compile()
    return nc


def _get_nc(s=S):
    with _LOCK:
        if s not in _CACHE:
            _CACHE[s] = _build(s)
        return _CACHE[s]


def _make_in_maps(inputs):
    hidden_states = np.asarray(inputs["hidden_states"], dtype=np.float32)
    attention_mask = np.asarray(inputs["attention_mask"], dtype=np.float32)
    Wq = np.asarray(inputs["Wq"], dtype=np.float32).astype(np.float16)
    Wk = np.asarray(inputs["Wk"], dtype=np.float32).astype(np.float16)
    Wv = np.asarray(inputs["Wv"], dtype=np.float32).astype(np.float16)
    bq = np.asarray(inputs["bq"], dtype=np.float32)
    bk = np.asarray(inputs["bk"], dtype=np.float32)
    bv = np.asarray(inputs["bv"], dtype=np.float32)

    in_maps = []
    for core in range(N_CORES):
        b, g = core // 2, core % 2
        js = slice(g * JC, (g + 1) * JC)
        in_maps.append({
            # fp16 + [HID, S] layout: the rounding the device would do on
            # load, plus the transpose the PE would otherwise compute
            "hT": np.ascontiguousarray(
                hidden_states[b].astype(np.float16).T),
            "mask": np.ascontiguousarray(attention_mask[b].reshape(S, 1)),
            "wq": np.ascontiguousarray(Wq[:, js]),
            "wk": np.ascontiguousarray(Wk[:, js]),
            "wv": np.ascontiguousarray(Wv[:, js]),
            "bq": np.ascontiguousarray(bq[js].reshape(JC, 1)),
            "bk": np.ascontiguousarray(bk[js].reshape(JC, 1)),
            "bv": np.ascontiguousarray(bv[js].reshape(1, JC)),
        })
    return in_maps


def kernel(hidden_states, attention_mask, Wq, bq, Wk, bk, Wv, bv):
    from concourse.bass_utils import run_bass_kernel_spmd

    nc = _get_nc()
    in_maps = _make_in_maps(dict(
        hidden_states=hidden_states, attention_mask=attention_mask,
        Wq=Wq, bq=bq, Wk=Wk, bk=bk, Wv=Wv, bv=bv))

    res = run_bass_kernel_spmd(nc, in_maps, core_ids=list(range(N_CORES)))
    out = np.empty((B, S, 16, D), dtype=np.float32)
    for core in range(N_CORES):
        b, g = core // 2, core % 2
        out[b, :, g * 8:(g + 1) * 8, :] = \
            res.results[core]["out"].reshape(S, 8, D)
    return out


# revision 23
# speedup vs baseline: 1.1358x; 1.0644x over previous
"""Trainium2 Bass kernel for nn_Attention_7009386627377.

Multi-head attention (16 heads, d=64) over [4, 2048, 1024] hidden states,
sharded across 8 NeuronCores as (batch b = core//2, head-group g = core%2 of
8 heads). Each core computes its disjoint [2048, 512] output slice with no
collectives; the host reassembles [4, 2048, 16, 64].

Per-core pipeline (fp16 compute, fp32 PSUM accumulation):
  The host pre-casts hidden/weights to fp16 (the device would round them to
  fp16 anyway) and pre-transposes hidden to the [HID, S] layout the PE
  needs, halving inbound DMA and removing all transpose matmuls/staging.
  DMA priority: wk + hT on HWDGE queues, wq -> wv on SWDGE, so attention
  starts ~10us in; K/Q projections stream between attention steps.
  qT is pre-scaled by EXPC1 so the DVE exp2 bit-trick is ONE tensor_scalar
  (i16 = sc + EXPC2, bitcast fp16); kt in {4,5,12,13} take that path, the
  rest use ScalarE exp ACTIVATE (scale folded). Row sums ride 4-up packed
  PE matmuls on PAIR-ADDED prob tiles (one DVE add per kt pair), halving
  their PE cost. Groups are software-pipelined in one flat (group, kt)
  stream with lag 3 so the next group's scores cover the previous group's
  drain+close (keeping the PE HAM-warm across boundaries). Closes go
  through a DRAM xbar-transpose round trip (all on the sync queue) except
  the last group, which transposes on the PE to shorten the tail.
"""
import threading

import numpy as np

B = 4
S = 2048
HID = 1024
JC = 512          # per-core qkv columns = 8 heads x 64
D = 64
N_CORES = 8

LOG2E = 1.4426950408889634
EXPC1 = 0.125 * LOG2E * 1024.0          # folded into qT at projection time
EXPC2 = 15360.0 - 44.0                  # fp16 exponent bias field + offset
SCALE_S = 0.125 / EXPC1                 # ScalarE exp scale (sc -> s_raw/8)
TRICK_KTS = (4, 5, 12, 13)              # kt tiles on the DVE exp2 path

_LOCK = threading.Lock()
_CACHE = {}


def _build(s=S):
    from contextlib import ExitStack

    from concourse import bacc, mybir
    import concourse.bass as bass
    import concourse.tile as tile
    from concourse.masks import make_identity

    F32 = mybir.dt.float32
    F16 = mybir.dt.float16
    I16 = mybir.dt.int16
    EXP = mybir.ActivationFunctionType.Exp
    COPY = mybir.ActivationFunctionType.Copy
    MUL = mybir.AluOpType.mult
    ADD = mybir.AluOpType.add
    SUB = mybir.AluOpType.subtract

    nst = s // 128           # s-tiles
    nq = max(1, s // 512)    # 512-wide quarters of s
    qw = s // nq             # quarter width
    nkt = s // 128           # key tiles

    nc = bacc.Bacc("TRN2", target_bir_lowering=False, debug=False,
                   enable_asserts=False)

    hT_d = nc.dram_tensor("hT", [HID, s], F16, kind="ExternalInput").ap()
    msk = nc.dram_tensor("mask", [s, 1], F32, kind="ExternalInput").ap()
    wq_d = nc.dram_tensor("wq", [HID, JC], F16, kind="ExternalInput").ap()
    wk_d = nc.dram_tensor("wk", [HID, JC], F16, kind="ExternalInput").ap()
    wv_d = nc.dram_tensor("wv", [HID, JC], F16, kind="ExternalInput").ap()
    bq_d = nc.dram_tensor("bq", [JC, 1], F32, kind="ExternalInput").ap()
    bk_d = nc.dram_tensor("bk", [JC, 1], F32, kind="ExternalInput").ap()
    bv_d = nc.dram_tensor("bv", [1, JC], F32, kind="ExternalInput").ap()
    out_d = nc.dram_tensor("out", [s, JC], F32, kind="ExternalOutput").ap()

    with tile.TileContext(nc) as tc, ExitStack() as ctx:
        P = ctx.enter_context
        persist = P(tc.tile_pool(name="persist", bufs=1))
        dram_pool = P(tc.tile_pool(name="dram", bufs=1, space="DRAM"))
        pt_pool = P(tc.tile_pool(name="pt", bufs=7))
        padd_pool = P(tc.tile_pool(name="padd", bufs=2))
        ctx_sb_pool = P(tc.tile_pool(name="ctxsb", bufs=2))
        sums_sb_pool = P(tc.tile_pool(name="sumssb", bufs=2))
        outt_pool = P(tc.tile_pool(name="outt", bufs=4))
        outf_pool = P(tc.tile_pool(name="outf", bufs=2))
        # PSUM: "big" = [128,1024] f32 scores slots (2 banks each, bufs=2),
        # "small" = [128,512] f32 slots for ctx/sums/V/projections (4 banks)
        ps_big = P(tc.tile_pool(name="psbig", bufs=2, space="PSUM"))
        ps_small = P(tc.tile_pool(name="pssmall", bufs=4, space="PSUM"))

        ident = persist.tile([128, 128], F16, tag="ident")
        make_identity(nc, ident[:])
        ones_rep = persist.tile([128, 32], F16, tag="ones_rep")
        nc.vector.memset(ones_rep[:], 1.0)
        ones_row = persist.tile([1, 128], F16, tag="ones_row")
        nc.vector.memset(ones_row[:], 1.0)

        # ---- DMA issue (sync/scalar = HWDGE, gpsimd = SWDGE) ----
        # wk + hT interleaved across both HWDGE queues; wq -> wv on SWDGE
        w_sb = {}
        hT = [persist.tile([128, s], F16, tag=f"hT{hc}", name=f"hT{hc}")
              for hc in range(8)]
        # biases/mask first, batched into single strided DMAs so the scalar
        # queue frees up in ~3us instead of serializing 25 tiny triggers
        bq_sb = persist.tile([128, 4], F32, tag="bq_sb")
        bk_sb = persist.tile([128, 4], F32, tag="bk_sb")
        nc.scalar.dma_start(bq_sb[:],
                            bq_d.rearrange("(p r) o -> r (p o)", p=4, r=128))
        nc.scalar.dma_start(bk_sb[:],
                            bk_d.rearrange("(p r) o -> r (p o)", p=4, r=128))
        bv_st = persist.tile([1, JC], F32, tag="bv_st")
        nc.scalar.dma_start(bv_st[:], bv_d[:, :])
        mask_sb = persist.tile([128, nst], F32, tag="mask_sb")
        nc.scalar.dma_start(
            mask_sb[:], msk.rearrange("(t p) o -> p (t o)", t=nst, p=128))
        for hc in range(8):
            wt = persist.tile([128, JC], F16, tag=f"wk{hc}")
            nc.scalar.dma_start(wt[:], wk_d[hc * 128:(hc + 1) * 128, :])
            w_sb[("wk", hc)] = wt
        # half-major hT chunks (all on sync) so the first K projections
        # (which contract over ALL hc) can start after ~2MB
        for half in range(2):
            hw = s // 2
            for hc in range(8):
                nc.sync.dma_start(
                    hT[hc][:, half * hw:(half + 1) * hw],
                    hT_d[hc * 128:(hc + 1) * 128, half * hw:(half + 1) * hw])
        for wname, wd in (("wq", wq_d), ("wv", wv_d)):
            for hc in range(8):
                wt = persist.tile([128, JC], F16, tag=f"{wname}{hc}")
                nc.gpsimd.dma_start(wt[:], wd[hc * 128:(hc + 1) * 128, :])
                w_sb[(wname, hc)] = wt
        bv_f16 = persist.tile([1, JC], F16, tag="bv_f16")
        nc.vector.tensor_copy(bv_f16[:], bv_st[:])
        # mb = (mask-1)*30: additive exp bias column per kt (0 for mask=1)
        mb = persist.tile([128, nst], F32, tag="mb")
        nc.vector.tensor_scalar(mb[:], mask_sb[:], 1.0, 30.0, SUB, MUL)

        qT = [persist.tile([128, s], F16, tag=f"qT{p}", name=f"qT{p}")
              for p in range(4)]
        kT = [persist.tile([128, s], F16, tag=f"kT{p}", name=f"kT{p}")
              for p in range(4)]
        v_sb = [persist.tile([128, JC], F16, tag=f"v{t}", name=f"v{t}")
                for t in range(nst)]
        scratch = dram_pool.tile([544, s], F16, tag="scratch")

        zrow = persist.tile([16, 512], F16, tag="zrow")
        nc.vector.memset(zrow[:], 0.0)
        for g in range(2):
            for zc in range(s // 512):
                nc.gpsimd.dma_start(
                    scratch[272 * g + 260:272 * g + 272,
                            zc * 512:(zc + 1) * 512], zrow[0:12, :])

        def project_k(p, sq):
            pp = ps_small.tile([128, qw], F32, tag="ps", name=f"ppk{p}_{sq}")
            for hc in range(8):
                nc.tensor.matmul(
                    pp[:], lhsT=w_sb[("wk", hc)][:, p * 128:(p + 1) * 128],
                    rhs=hT[hc][:, sq * qw:(sq + 1) * qw],
                    start=(hc == 0), stop=(hc == 7))
            nc.vector.tensor_scalar(kT[p][:, sq * qw:(sq + 1) * qw],
                                    pp[:], bk_sb[:, p:p + 1], None, ADD)

        def project_q(p, sq):
            # bias add + EXPC1 prescale folded into the psum->sbuf copy
            pp = ps_small.tile([128, qw], F32, tag="ps", name=f"ppq{p}_{sq}")
            for hc in range(8):
                nc.tensor.matmul(
                    pp[:], lhsT=w_sb[("wq", hc)][:, p * 128:(p + 1) * 128],
                    rhs=hT[hc][:, sq * qw:(sq + 1) * qw],
                    start=(hc == 0), stop=(hc == 7))
            nc.vector.tensor_scalar(qT[p][:, sq * qw:(sq + 1) * qw],
                                    pp[:], bq_sb[:, p:p + 1], EXPC1,
                                    ADD, MUL)

        def produce_v(st):
            # V for s-tile st (+bias via K=1 matmul, mask fold on the copy)
            vp = ps_small.tile([128, JC], F32, tag="ps", name=f"vp{st}")
            for hc in range(8):
                nc.tensor.matmul(vp[:],
                                 lhsT=hT[hc][:, st * 128:(st + 1) * 128],
                                 rhs=w_sb[("wv", hc)][:],
                                 start=(hc == 0), stop=False)
            nc.tensor.matmul(vp[:], lhsT=ones_row[:], rhs=bv_f16[:],
                             start=False, stop=True)
            nc.scalar.activation(v_sb[st][:], vp[:], COPY,
                                 scale=mask_sb[:, st:st + 1])

        # ---- attention: flat (group, kt) stream, lag-3 pipeline ----
        class Group:
            def __init__(g, q, r):
                g.q, g.r = q, r
                g.qs = slice(q * qw, (q + 1) * qw)
                g.pA, g.pB = 2 * r, 2 * r + 1
                g.ctxA = None
                g.prev = None

            def alloc(g):
                g.ctxA = ps_small.tile([128, qw], F32, tag="ps",
                                       name=f"ctxA{g.q}_{g.r}")
                g.ctxB = ps_small.tile([128, qw], F32, tag="ps",
                                       name=f"ctxB{g.q}_{g.r}")
                g.sums = ps_small.tile([128, qw], F32, tag="ps",
                                       name=f"sums{g.q}_{g.r}")

            def scores_exp(g, kt):
                ks = slice(kt * 128, (kt + 1) * 128)
                # one [128, 2048] pt tile per step (halves = head pairs
                # A/B) so the later pair-add is a single DVE op. Tricked
                # kts allocate i16 and are bitcast-read as fp16.
                tricked = kt in TRICK_KTS
                ptt = pt_pool.tile([128, 4 * qw], I16 if tricked else F16,
                                   tag="pt")
                pts = []
                for i, ppp in enumerate((g.pA, g.pB)):
                    sc = ps_big.tile([128, 2 * qw], F32, tag="big")
                    nc.tensor.matmul(sc[:, 0:qw], lhsT=kT[ppp][0:64, ks],
                                     rhs=qT[ppp][0:64, g.qs],
                                     start=True, stop=True,
                                     skip_group_check=True,
                                     tile_position=(0, 0))
                    nc.tensor.matmul(sc[:, qw:2 * qw],
                                     lhsT=kT[ppp][64:128, ks],
                                     rhs=qT[ppp][64:128, g.qs],
                                     start=True, stop=True,
                                     skip_group_check=True,
                                     tile_position=(64, 0))
                    half = ptt[:, i * 2 * qw:(i + 1) * 2 * qw]
                    if tricked:
                        # DVE exp2 bit-trick: one ALU op (qT pre-scaled)
                        nc.vector.tensor_scalar(half, sc[:], EXPC2, None,
                                                ADD)
                        pts.append(half.bitcast(F16))
                    else:
                        nc.scalar.activation(half, sc[:], EXP,
                                             scale=SCALE_S,
                                             bias=mb[:, kt:kt + 1])
                        pts.append(half)
                return [ptt, *pts]

            def ctx_sums(g, kt, ptt, ptA, ptB):
                if g.ctxA is None:
                    g.alloc()
                for ppp, ctx_ps, pt in ((g.pA, g.ctxA, ptA),
                                        (g.pB, g.ctxB, ptB)):
                    nc.tensor.matmul(
                        ctx_ps[0:64, :],
                        lhsT=v_sb[kt][:, ppp * 128:ppp * 128 + 64],
                        rhs=pt[:, 0:qw], start=(kt == 0),
                        stop=(kt == nkt - 1), skip_group_check=True,
                        tile_position=(0, 0))
                    nc.tensor.matmul(
                        ctx_ps[64:128, :],
                        lhsT=v_sb[kt][:, ppp * 128 + 64:ppp * 128 + 128],
                        rhs=pt[:, qw:2 * qw], start=(kt == 0),
                        stop=(kt == nkt - 1), skip_group_check=True,
                        tile_position=(0, 64))
                if kt % 2 == 0:
                    g.prev = ptt
                    return
                # pair-added prob tiles halve the 4-up sums matmul rate;
                # one [128, 4*qw] DVE add covers both head pairs
                pa = padd_pool.tile([128, 4 * qw], F16, tag="padd")
                nc.vector.tensor_tensor(
                    pa[:], g.prev[:].bitcast(F16), ptt[:].bitcast(F16), ADD)
                g.prev = None
                j = kt // 2
                for i, pa_half in enumerate(
                        (pa[:, 0:qw], pa[:, qw:2 * qw],
                         pa[:, 2 * qw:3 * qw], pa[:, 3 * qw:4 * qw])):
                    nc.tensor.matmul(
                        g.sums[32 * i:32 * (i + 1), :], lhsT=ones_rep[:],
                        rhs=pa_half, start=(j == 0),
                        stop=(j == nkt // 2 - 1), skip_group_check=True,
                        tile_position=(0, 32 * i))

            def close_write(g):
                # copies + scratch writes + xbar transpose issue; the
                # normalize waits on xbar data, so it's DEFERRED (a few
                # steps later) to keep the in-order DVE from stalling and
                # holding up the next group's padds/exps behind it.
                q, r = g.q, g.r
                base = 272 * r
                for gi, ctx_ps in ((0, g.ctxA), (1, g.ctxB)):
                    ctx_sb = ctx_sb_pool.tile([128, qw], F16, tag="ctxsb")
                    nc.vector.tensor_copy(ctx_sb[:], ctx_ps[:])
                    nc.sync.dma_start(
                        scratch[base + gi * 128:base + (gi + 1) * 128, g.qs],
                        ctx_sb[:])
                ssb = sums_sb_pool.tile([128, qw], F16, tag="sumssb")
                for i in range(4):
                    nc.vector.tensor_copy(ssb[32 * i:32 * i + 1, :],
                                          g.sums[32 * i:32 * i + 1, :])
                    nc.sync.dma_start(
                        scratch[base + 256 + i:base + 257 + i, g.qs],
                        ssb[32 * i:32 * i + 1, :])
                g.ots = []
                for b4 in range(qw // 128):
                    sbg = q * (qw // 128) + b4
                    ot = outt_pool.tile([128, 272], F16, tag="outt")
                    nc.sync.dma_start_transpose(
                        ot[:], scratch[base:base + 272,
                                       sbg * 128:(sbg + 1) * 128])
                    g.ots.append(ot)

            def close_finish(g):
                q, r = g.q, g.r
                for b4 in range(qw // 128):
                    sbg = q * (qw // 128) + b4
                    ot = g.ots[b4]
                    rc = persist.tile([128, 4], F32, tag=f"rc{sbg}_{r}",
                                      name=f"rc{sbg}_{r}")
                    nc.vector.reciprocal(rc[:], ot[:, 256:260])
                    of = outf_pool.tile([128, 256], F32, tag="outf")
                    for h in range(4):
                        nc.vector.tensor_scalar(
                            of[:, h * D:(h + 1) * D],
                            ot[:, h * D:(h + 1) * D],
                            rc[:, h:h + 1], None, MUL)
                    nc.sync.dma_start(
                        out_d[sbg * 128:(sbg + 1) * 128,
                              r * 256:(r + 1) * 256], of[:])

            def close_fast(g):
                # last group: transpose ctx/sums on the PE instead of the
                # DMA xbar round trip through DRAM — shortens the tail.
                q, r = g.q, g.r
                csA = ctx_sb_pool.tile([128, qw], F16, tag="ctxsb")
                nc.scalar.activation(csA[:], g.ctxA[:], COPY)
                csB = ctx_sb_pool.tile([128, qw], F16, tag="ctxsb")
                nc.vector.tensor_copy(csB[:], g.ctxB[:])
                # zero first: the identity-matmul transpose reads ALL 128
                # rows, and NaN bit-patterns in garbage rows would poison
                # every output column (NaN * 0 = NaN)
                ssb = sums_sb_pool.tile([128, qw], F16, tag="sumssb")
                nc.vector.memset(ssb[:], 0.0)
                for i in range(4):
                    nc.vector.tensor_copy(ssb[32 * i:32 * i + 1, :],
                                          g.sums[32 * i:32 * i + 1, :])
                for b4 in range(qw // 128):
                    sbg = q * (qw // 128) + b4
                    cs = slice(b4 * 128, (b4 + 1) * 128)
                    tpo = ps_big.tile([128, 2 * qw], F32, tag="big")
                    nc.tensor.matmul(tpo[:, 0:128], lhsT=csA[:, cs],
                                     rhs=ident[:], start=True, stop=True,
                                     skip_group_check=True)
                    nc.tensor.matmul(tpo[:, 128:256], lhsT=csB[:, cs],
                                     rhs=ident[:], start=True, stop=True,
                                     skip_group_check=True)
                    nc.tensor.matmul(tpo[:, 512:640], lhsT=ssb[:, cs],
                                     rhs=ident[:], start=True, stop=True,
                                     skip_group_check=True)
                    rc = persist.tile([128, 4], F32, tag=f"rcf{sbg}",
                                      name=f"rcf{sbg}")
                    for i in range(4):
                        nc.vector.reciprocal(
                            rc[:, i:i + 1],
                            tpo[:, 512 + 32 * i:513 + 32 * i])
                    of = outf_pool.tile([128, 256], F32, tag="outf")
                    for h in range(4):
                        nc.vector.tensor_scalar(
                            of[:, h * D:(h + 1) * D],
                            tpo[:, (h % 2) * D + (h // 2) * 128:
                                (h % 2) * D + (h // 2) * 128 + D],
                            rc[:, h:h + 1], None, MUL)
                    nc.sync.dma_start(
                        out_d[sbg * 128:(sbg + 1) * 128,
                              r * 256:(r + 1) * 256], of[:])

        # ---- upfront: just enough projections for g00's first steps ----
        project_k(0, 0)
        project_k(1, 0)
        project_q(0, 0)
        project_q(1, 0)

        groups = [Group(q, r) for q in range(nq) for r in range(2)]

        # injection schedule: gidx -> kt -> list of thunks
        inject = {gi: {} for gi in range(len(groups))}

        def add(gi, kt, fn, *a):
            inject[gi].setdefault(kt, []).append((fn, a))

        # g00 streams the remaining K projections just ahead of use
        for kt, (p, sq) in enumerate((
                (0, 1), (1, 1), (2, 0), (3, 0),
                (0, 2), (1, 2), (2, 1), (3, 1),
                (0, 3), (1, 3), (2, 2), (3, 2),
                (2, 3), (3, 3))):
            add(0, kt, project_k, p, sq)
        add(0, 13, project_q, 2, 0)
        add(0, 14, project_q, 3, 0)
        # group (q,0) q>=1 injects Q for (q,1); (q,1) injects Q for (q+1,0)
        for q in range(nq):
            gi_r0, gi_r1 = 2 * q, 2 * q + 1
            if q >= 1:
                add(gi_r0, 5, project_q, 2, q)
                add(gi_r0, 10, project_q, 3, q)
            if q + 1 < nq:
                add(gi_r1, 6, project_q, 0, q + 1)
                add(gi_r1, 10, project_q, 1, q + 1)

        pend = []
        todo = []

        def tick_todo():
            for item in list(todo):
                item[0] -= 1
                if item[0] <= 0:
                    todo.remove(item)
                    item[1]()

        def drain_one():
            g2, kt2, pts2 = pend.pop(0)
            g2.ctx_sums(kt2, *pts2)
            if kt2 == nkt - 1 and g2 is not groups[-1]:
                g2.close_write()
                todo.append([5, lambda gg=g2: gg.close_finish()])

        for gi, g in enumerate(groups):
            for kt in range(nkt):
                for fn, a in inject[gi].get(kt, ()):
                    fn(*a)
                pts = g.scores_exp(kt)
                if gi == 0:
                    produce_v(kt)
                pend.append((g, kt, pts))
                tick_todo()
                if len(pend) > 3:
                    drain_one()
        while pend:
            drain_one()
        for item in todo:
            item[1]()
        groups[-1].close_fast()

    nc.compile()
    return nc


def _get_nc(s=S):
    with _LOCK:
        if s not in _CACHE:
            _CACHE[s] = _build(s)
        return _CACHE[s]


def _make_in_maps(inputs):
    hidden_states = np.asarray(inputs["hidden_states"], dtype=np.float32)
    attention_mask = np.asarray(inputs["attention_mask"], dtype=np.float32)
    Wq = np.asarray(inputs["Wq"], dtype=np.float32).astype(np.float16)
    Wk = np.asarray(inputs["Wk"], dtype=np.float32).astype(np.float16)
    Wv = np.asarray(inputs["Wv"], dtype=np.float32).astype(np.float16)
    bq = np.asarray(inputs["bq"], dtype=np.float32)
    bk = np.asarray(inputs["bk"], dtype=np.float32)
    bv = np.asarray(inputs["bv"], dtype=np.float32)

    in_maps = []
    for core in range(N_CORES):
        b, g = core // 2, core % 2
        js = slice(g * JC, (g + 1) * JC)
        in_maps.append({
            # fp16 + [HID, S] layout: the rounding the device would do on
            # load, plus the transpose the PE would otherwise compute
            "hT": np.ascontiguousarray(
                hidden_states[b].astype(np.float16).T),
            "mask": np.ascontiguousarray(attention_mask[b].reshape(S, 1)),
            "wq": np.ascontiguousarray(Wq[:, js]),
            "wk": np.ascontiguousarray(Wk[:, js]),
            "wv": np.ascontiguousarray(Wv[:, js]),
            "bq": np.ascontiguousarray(bq[js].reshape(JC, 1)),
            "bk": np.ascontiguousarray(bk[js].reshape(JC, 1)),
            "bv": np.ascontiguousarray(bv[js].reshape(1, JC)),
        })
    return in_maps


def kernel(hidden_states, attention_mask, Wq, bq, Wk, bk, Wv, bv):
    from concourse.bass_utils import run_bass_kernel_spmd

    nc = _get_nc()
    in_maps = _make_in_maps(dict(
        hidden_states=hidden_states, attention_mask=attention_mask,
        Wq=Wq, bq=bq, Wk=Wk, bk=bk, Wv=Wv, bv=bv))

    res = run_bass_kernel_spmd(nc, in_maps, core_ids=list(range(N_CORES)))
    out = np.empty((B, S, 16, D), dtype=np.float32)
    for core in range(N_CORES):
        b, g = core // 2, core % 2
        out[b, :, g * 8:(g + 1) * 8, :] = \
            res.results[core]["out"].reshape(S, 8, D)
    return out
